# revision 9
# baseline (speedup 1.0000x reference)
"""Multi-head attention Bass/Tile kernel for Trainium2, 8-core SPMD — v3.

v3 over v2: phase D is ACT(exp)-bound, so the PE queue is software-pipelined
(next score tile issues before the current PV pair), the per-head Ln/Exp
normalization is batched into one [8,512] Ln + Exp per query chunk (den rows
gathered by tiny PSUM->SBUF DMAs, reciprocal broadcast via a one-hot selector
matmul), and the K/Q projections for later head-pairs plus the ic0 output
projection are interleaved into the attention loops to fill PE slack.
"""

import sys

if "/opt/trn_rl_repo" not in sys.path:
    sys.path.insert(0, "/opt/trn_rl_repo")

from contextlib import ExitStack

import numpy as np
import ml_dtypes

_BF16NP = ml_dtypes.bfloat16

import concourse.bass as bass
import concourse.tile as tile
from concourse import mybir
import bass_rust as _bass_rust

F32 = mybir.dt.float32
F32R = mybir.dt.float32r
BF16 = mybir.dt.bfloat16
EXP = mybir.ActivationFunctionType.Exp
LN = mybir.ActivationFunctionType.Ln

B, Q, KL, D, H = 4, 2048, 2048, 512, 8
HD = D // H            # 64
QS = Q // 2            # 1024 query rows per core
SCALE = 1.0 / HD ** 0.5
EXPBIAS = -30.0


def _legalize_waits(nc, max_waits=1):
    n = 0
    for f in nc.m.functions:
        for bb in f.blocks:
            insts = bb.instructions
            i = 0
            while i < len(insts):
                inst = insts[i]
                si = inst.sync_info
                if si is not None and len(si.on_wait) > max_waits:
                    waits = list(si.on_wait)
                    for j, w in enumerate(waits[max_waits:]):
                        nop = mybir.InstNoOp(
                            name=f"{inst.name}-waitsplit{j}", ins=[], outs=[]
                        )
                        nop.engine = inst.engine
                        nop.sync_info = _bass_rust.SyncInfo(on_wait=[w], on_update=[])
                        insts.insert(i, nop)
                        i += 1
                        n += 1
                    inst.sync_info = _bass_rust.SyncInfo(
                        on_wait=waits[:max_waits], on_update=list(si.on_update)
                    )
                i += 1
    return n


def build_kernel(KLE):
    NJT = KLE // 128
    nc = bass.Bass("TRN2", target_bir_lowering=False, debug=False)

    qT_d = nc.dram_tensor("qT", [D, QS], F32R, kind="ExternalInput").ap()
    kT_d = nc.dram_tensor("kT", [D, KLE], F32R, kind="ExternalInput").ap()
    vT_d = nc.dram_tensor("vT", [D, KLE], BF16, kind="ExternalInput").ap()
    w_d = {
        w: nc.dram_tensor(w, [D, D], F32R, kind="ExternalInput").ap()
        for w in ("wqT", "wkT", "woT")
    }
    w_d["wvT"] = nc.dram_tensor("wvT", [D, D], BF16, kind="ExternalInput").ap()
    m_d = nc.dram_tensor("mask2d", [128, NJT], F32, kind="ExternalInput").ap()
    out_d = nc.dram_tensor("out", [QS, D], F32, kind="ExternalOutput").ap()

    # one-hot selector: sel[p, hp*128 + m] = 1 if p == 2*hp + (m >= 64)
    sel_np = np.zeros((8, 512), np.float32)
    for hp in range(4):
        sel_np[2 * hp, hp * 128:hp * 128 + 64] = 1.0
        sel_np[2 * hp + 1, hp * 128 + 64:hp * 128 + 128] = 1.0
    sel_d = nc.inline_tensor(sel_np, name="sel")

    with tile.TileContext(nc) as tc, ExitStack() as ctx:
        pc = ctx.enter_context(tc.tile_pool(name="const", bufs=1))
        m_sb = pc.tile([128, NJT], F32, tag="m_sb")
        nc.sync.dma_start(m_sb[:], m_d)
        ebias = pc.tile([128, 1], F32, tag="ebias")
        nc.vector.memset(ebias[:], EXPBIAS)
        sel_f = pc.tile([8, 512], F32, tag="sel_f")
        nc.sync.dma_start(sel_f[:], sel_d.ap())
        sel = pc.tile([8, 512], F32R, tag="sel")
        nc.vector.tensor_copy(sel[:], sel_f[:])

        # ---- input tiles (DMA direct, pre-transposed on host) -----------
        pin = ctx.enter_context(tc.tile_pool(name="inputs", bufs=1))
        wsb = {}
        for w in ("wkT", "wqT", "woT"):
            wsb[w] = [pin.tile([128, D], F32R, tag=f"{w}{i}", name=f"{w}{i}") for i in range(4)]
        wsb["wvT"] = [pin.tile([128, D], BF16, tag=f"wvT{i}", name=f"wvT{i}") for i in range(4)]
        kT = [pin.tile([128, KLE], F32R, tag=f"kTi{i}", name=f"kTi{i}") for i in range(4)]
        qT = [pin.tile([128, QS], F32R, tag=f"qTi{i}", name=f"qTi{i}") for i in range(4)]
        vT = [pin.tile([128, KLE], BF16, tag=f"vTi{i}", name=f"vTi{i}") for i in range(4)]
        # DMA emission order ~ earliest consumer: scores need wq/q-half/wk/k,
        # then the V path streams in per column chunk just ahead of its PV use
        for dk in range(4):
            nc.sync.dma_start(wsb["wqT"][dk][:], w_d["wqT"].rearrange("(t p) d -> t p d", p=128)[dk])
        for dk in range(4):
            nc.sync.dma_start(qT[dk][:, 0:512], qT_d.rearrange("(t p) d -> t p d", p=128)[dk][:, 0:512])
        for dk in range(4):
            nc.sync.dma_start(wsb["wkT"][dk][:], w_d["wkT"].rearrange("(t p) d -> t p d", p=128)[dk])
        for c0 in range(0, KLE, 512):
            cw = min(512, KLE - c0)
            for dk in range(4):
                nc.sync.dma_start(
                    kT[dk][:, c0:c0 + cw],
                    kT_d.rearrange("(t p) d -> t p d", p=128)[dk][:, c0:c0 + cw],
                )
        for dk in range(4):
            nc.sync.dma_start(wsb["wvT"][dk][:], w_d["wvT"].rearrange("(t p) d -> t p d", p=128)[dk])
        for c0 in range(0, KLE, 384):
            cw = min(384, KLE - c0)
            for dk in range(4):
                nc.sync.dma_start(
                    vT[dk][:, c0:c0 + cw],
                    vT_d.rearrange("(t p) d -> t p d", p=128)[dk][:, c0:c0 + cw],
                )
        for dk in range(4):
            nc.sync.dma_start(qT[dk][:, 512:1024], qT_d.rearrange("(t p) d -> t p d", p=128)[dk][:, 512:1024])
        for dk in range(4):
            nc.sync.dma_start(wsb["woT"][dk][:], w_d["woT"].rearrange("(t p) d -> t p d", p=128)[dk])

        pp = ctx.enter_context(tc.tile_pool(name="proj", bufs=1))
        KT = [pp.tile([128, KLE], F32R, tag=f"KT{i}", name=f"KT{i}") for i in range(4)]
        QT = [pp.tile([128, QS], F32R, tag=f"QT{i}", name=f"QT{i}") for i in range(4)]
        VS = [pp.tile([128, H * (HD + 1)], BF16, tag=f"VS{i}", name=f"VS{i}") for i in range(NJT)]
        pA = ctx.enter_context(tc.tile_pool(name="attn_out", bufs=1))
        A2r = [pA.tile([128, QS], F32, tag=f"A2r{hp}", name=f"A2r{hp}") for hp in range(4)]
        A2 = [pA.tile([128, QS], F32R, tag=f"A2{hp}", name=f"A2{hp}") for hp in range(4)]

        with tc.tile_pool(name="eP", bufs=6) as pe_pool, \
             tc.tile_pool(name="rP", bufs=2) as pr, \
             tc.tile_pool(name="bP", bufs=2) as pb, \
             tc.tile_pool(name="psumS", bufs=2, space="PSUM") as ppsS, \
             tc.tile_pool(name="psumPV", bufs=2, space="PSUM") as ppsPV, \
             tc.tile_pool(name="psumX", bufs=2, space="PSUM") as ppsX:

            # ---- deferred work-item machinery (PE filler groups) --------
            nevac = [0]

            def evac(dst, src, eng="v"):
                if eng == "v":
                    nc.vector.tensor_copy(dst, src)
                else:
                    nc.scalar.copy(dst, src)

            # accumulation chains interleaved across two PSUM tiles so one
            # chain's drain overlaps the other's stream
            def _mm_pair(specs):
                tiles = [
                    ppsX.tile([128, 512], F32, tag="aux", name=f"ps_{nm}")
                    for nm, _, _, _ in specs
                ]
                for dk in range(4):
                    for t, (nm, w, lhs_fn, rhs_fn) in zip(tiles, specs):
                        nc.tensor.matmul(
                            t[:, 0:w],
                            lhs_fn(dk), rhs_fn(dk),
                            start=(dk == 0), stop=(dk == 3),
                        )
                return tiles

            def k_spec(ot, j0):
                jw = min(512, KLE - j0)
                return (f"k{ot}_{j0}", jw,
                        lambda dk: wsb["wkT"][dk][:, ot * 128:(ot + 1) * 128],
                        lambda dk: kT[dk][:, j0:j0 + jw])

            def q_spec(ot, icc):
                return (f"q{ot}_{icc}", 512,
                        lambda dk: wsb["wqT"][dk][:, ot * 128:(ot + 1) * 128],
                        lambda dk: qT[dk][:, icc * 512:(icc + 1) * 512])

            def v_spec(jt):
                return (f"v{jt}", 512,
                        lambda dk: vT[dk][:, jt * 128:(jt + 1) * 128],
                        lambda dk: wsb["wvT"][dk][:])

            def k_evac(ot, j0, ps, eng):
                jw = min(512, KLE - j0)
                evac(KT[ot][:, j0:j0 + jw], ps[:, 0:jw], eng)

            def q_evac(ot, icc, ps, eng):
                evac(QT[ot][:, icc * 512:(icc + 1) * 512], ps[:], eng)

            def v_evac(jt, ps):
                vs_out = VS[jt][:].rearrange("p (h d) -> p h d", d=HD + 1)
                nc.vector.tensor_scalar(
                    vs_out[:, :, 0:HD],
                    ps[:].rearrange("p (h d) -> p h d", d=HD),
                    m_sb[:, jt:jt + 1],
                    None,
                    mybir.AluOpType.mult,
                )
                nc.vector.tensor_copy(
                    vs_out[:, :, HD].squeeze(),
                    m_sb[:, jt:jt + 1].broadcast_to([128, H]),
                )

            def wo_group(itl, ic):
                # two query sub-chunks with interleaved accumulation chains
                tiles = []
                for half in range(2):
                    tiles.append(ppsX.tile([128, D], F32, tag="aux", name=f"ops{ic}_{itl}_{half}"))
                for hp in range(4):
                    for half, o_ps in enumerate(tiles):
                        c0 = ic * 512 + itl * 256 + half * 128
                        nc.tensor.matmul(
                            o_ps[:],
                            A2[hp][:, c0:c0 + 128],
                            wsb["woT"][hp][:],
                            start=(hp == 0),
                            stop=(hp == 3),
                        )
                for half, o_ps in enumerate(tiles):
                    c0 = ic * 512 + itl * 256 + half * 128
                    o_sb = pb.tile([128, D], F32, tag="osb", name=f"osb{ic}_{itl}_{half}")
                    nc.vector.tensor_copy(o_sb[:], o_ps[:])
                    nc.sync.dma_start(out_d[c0:c0 + 128, :], o_sb[:])

            # ---- prefix: only the two chains the first scores need ------
            t = _mm_pair([q_spec(0, 0)])
            q_evac(0, 0, t[0], "s")
            t = _mm_pair([k_spec(0, 0)])
            k_evac(0, 0, t[0], "s")

            # single filler queue of (spec, evac) groups, popped in pairs;
            # with hp-outer/ic-inner order the work spreads over both query
            # chunks of each head-pair
            filler = []
            for j0 in range(512, KLE, 512):
                filler.append(("k", 0, j0))
            for jt in range(NJT):
                filler.append(("v", jt, 0))
            filler.append(("q", 0, 1))
            for ot in range(1, 4):
                filler.append(("q", ot, 0))
                for j0 in range(0, KLE, 512):
                    filler.append(("k", ot, j0))
                filler.append(("q", ot, 1))
            for itl in range(2):
                filler.append(("wo", itl, 0))

            def run_filler(hp, jt):
                # V groups must stay ahead of their PV consumer; K/Q groups
                # for ot must land before head-pair ot starts; wo only after
                # the ic0 normalization has fired (gated by caller order)
                if filler and filler[0][0] == "wo" and not wo_ready[0]:
                    return
                pair = []
                while filler and len(pair) < 2:
                    kind, a, b = filler[0]
                    if kind in ("k", "q") and a > hp + 1:
                        break
                    if kind == "wo" and len(pair) == 1:
                        break  # wo groups emit singly (own psum + dma)
                    pair.append(filler.pop(0))
                    if pair[0][0] == "wo":
                        break
                if not pair:
                    return
                if pair[0][0] == "wo":
                    wo_group(pair[0][1], pair[0][2])
                    return
                specs = []
                for kind, a, b in pair:
                    specs.append(k_spec(a, b) if kind == "k" else
                                 q_spec(a, b) if kind == "q" else v_spec(a))
                ts = _mm_pair(specs)
                for (kind, a, b), ps in zip(pair, ts):
                    if kind == "k":
                        k_evac(a, b, ps, "v")
                    elif kind == "q":
                        q_evac(a, b, ps, "v")
                    else:
                        v_evac(a, ps)

            # ---- attention ----------------------------------------------
            def norm_emit(ic, den_g, hps, row0):
                # batched reciprocal + broadcast + normalize for `hps`;
                # den rows for hps[k] live at den_g[row0 + 2k : row0 + 2k+2]
                i0 = ic * 512
                nr = 2 * len(hps)
                ln_g = pr.tile([8, 512], F32, tag="lng", name=f"lng{ic}_{hps[0]}")
                r_g_ = pr.tile([8, 512], F32R, tag="rg", name=f"rg{ic}_{hps[0]}")
                nc.scalar.activation(ln_g[0:nr, :], den_g[row0:row0 + nr, :], LN)
                nc.scalar.activation(r_g_[0:nr, :], ln_g[0:nr, :], EXP, scale=-1.0)
                for k, hp in enumerate(hps):
                    bc = ppsX.tile([128, 512], F32, tag="aux", name=f"bc{hp}_{ic}")
                    # sel rows 2k,2k+1 with the k-th pair's one-hot pattern
                    nc.tensor.matmul(
                        bc[:],
                        sel[0:nr, k * 128:(k + 1) * 128],
                        r_g_[0:nr, :],
                        start=True, stop=True,
                    )
                    nc.vector.tensor_mul(
                        A2[hp][:, i0:i0 + 512], A2r[hp][:, i0:i0 + 512], bc[:]
                    )

            pending_norm = [None]
            wo_ready = [False]
            den_gs = [
                pr.tile([8, 512], F32, tag="deng", name="deng0"),
                pr.tile([8, 512], F32, tag="deng1", name="deng1"),
            ]
            den_g2 = pr.tile([8, 512], F32, tag="deng2", name="deng2")
            for hp in range(4):
                for ic in range(2):
                    i0 = ic * 512
                    den_g = den_gs[ic]
                    he, ho = 2 * hp, 2 * hp + 1
                    den_t, dr = (den_g2, 0) if (ic == 1 and hp == 3) else (den_g, 2 * hp)
                    pv_e = ppsPV.tile([65, 512], F32, tag="pv", name=f"pve{hp}_{ic}")
                    pv_o = ppsPV.tile([65, 512], F32, tag="pv", name=f"pvo{hp}_{ic}")

                    def s_mm(jt):
                        s_ps = ppsS.tile([128, 1024], F32, tag="s", name=f"s{hp}_{ic}_{jt}")
                        for po2, sl in ((0, slice(0, 512)), (HD, slice(512, 1024))):
                            nc.tensor.matmul(
                                s_ps[:, sl],
                                KT[hp][po2:po2 + HD, jt * 128:(jt + 1) * 128],
                                QT[hp][po2:po2 + HD, i0:i0 + 512],
                                start=True, stop=True,
                            )
                        return s_ps

                    s_cur = s_mm(0)
                    for jt in range(NJT):
                        e_t = pe_pool.tile([128, 1024], BF16, tag="e", name=f"e{hp}_{ic}_{jt}")
                        nc.scalar.activation(e_t[:], s_cur[:], EXP, scale=SCALE, bias=ebias[:, 0:1])
                        if jt + 1 < NJT:
                            s_cur = s_mm(jt + 1)
                        if hp == 0:
                            run_filler(hp, jt)
                            if jt == 0:
                                run_filler(hp, jt)
                        if jt == 1 and pending_norm[0] is not None:
                            pending_norm[0]()
                            pending_norm[0] = None
                        nc.tensor.matmul(
                            pv_e[0:65, :],
                            VS[jt][:, he * (HD + 1):(he + 1) * (HD + 1)],
                            e_t[:, 0:512],
                            start=(jt == 0), stop=(jt == NJT - 1),
                        )
                        nc.tensor.matmul(
                            pv_o[0:65, :],
                            VS[jt][:, ho * (HD + 1):(ho + 1) * (HD + 1)],
                            e_t[:, 512:1024],
                            start=(jt == 0), stop=(jt == NJT - 1),
                        )
                        if hp != 0 and jt % 2 == 1:
                            run_filler(hp, jt)
                    # evacuate raw pair + gather denominators; frees pv slots
                    nc.vector.tensor_copy(A2r[hp][0:HD, i0:i0 + 512], pv_e[0:HD, :])
                    nc.vector.tensor_copy(A2r[hp][HD:128, i0:i0 + 512], pv_o[0:HD, :])
                    # DMA can't read PSUM: bounce den rows through partition 64
                    # of an SBUF stage, then cross-partition SBUF->SBUF DMA
                    stage = pb.tile([65, 1024], F32, tag="dstage", name=f"dst{hp}_{ic}")
                    nc.vector.tensor_copy(stage[64:65, 0:512], pv_e[64:65, :])
                    nc.vector.tensor_copy(stage[64:65, 512:1024], pv_o[64:65, :])
                    nc.sync.dma_start(den_t[dr:dr + 1, :], stage[64:65, 0:512])
                    nc.sync.dma_start(den_t[dr + 1:dr + 2, :], stage[64:65, 512:1024])
                    run_filler(hp, -1)
                    if hp == 2 and ic == 1:
                        # ic1 hp0-2 normalize during (hp3, ic0)'s loop
                        pending_norm[0] = (
                            lambda d=den_gs[1]: norm_emit(1, d, [0, 1, 2], 0))
                    if hp == 3 and ic == 0:
                        # ic0 full normalize during (hp3, ic1)'s loop; output
                        # projection for ic0 becomes runnable right after
                        def _norm0(d=den_gs[0]):
                            norm_emit(0, d, [0, 1, 2, 3], 0)
                            wo_ready[0] = True
                        pending_norm[0] = _norm0
            # tail: last head-pair normalization + second-chunk output proj
            norm_emit(1, den_g2, [3], 0)
            for itl in range(2):
                wo_group(itl, 1)

    return nc


_NC_CACHE = {}


def _get_nc(KLE):
    if KLE not in _NC_CACHE:
        nc = build_kernel(KLE)
        _legalize_waits(nc)
        _NC_CACHE[KLE] = nc
    return _NC_CACHE[KLE]


def shard_inputs(query, key, value, Wq, Wk, Wv, Wo, attn_mask):
    idxs = [np.nonzero(np.asarray(attn_mask[b]) != 0)[0] for b in range(B)]
    maxcnt = max((len(ix) for ix in idxs), default=1)
    KLE = max(128, -(-maxcnt // 128) * 128)
    wqT = np.ascontiguousarray(np.asarray(Wq, np.float32).T)
    wkT = np.ascontiguousarray(np.asarray(Wk, np.float32).T)
    wvT = np.ascontiguousarray(np.asarray(Wv, np.float32).T.astype(_BF16NP))
    woT = np.ascontiguousarray(np.asarray(Wo, np.float32).T)
    in_maps = []
    for c in range(8):
        b, half = c // 2, c % 2
        idx = idxs[b]
        kc = np.zeros((D, KLE), np.float32)
        vc = np.zeros((D, KLE), _BF16NP)
        kc[:, : len(idx)] = np.asarray(key[b], np.float32)[idx].T
        vc[:, : len(idx)] = np.asarray(value[b], np.float32)[idx].T.astype(_BF16NP)
        mf = np.zeros(KLE, np.float32)
        mf[: len(idx)] = 1.0
        in_maps.append({
            "qT": np.ascontiguousarray(
                np.asarray(query[b, half * QS:(half + 1) * QS], np.float32).T
            ),
            "kT": kc,
            "vT": vc,
            "wqT": wqT, "wkT": wkT, "wvT": wvT, "woT": woT,
            "mask2d": np.ascontiguousarray(mf.reshape(KLE // 128, 128).T),
        })
    return in_maps, KLE


def kernel(query, key, value, Wq, Wk, Wv, Wo, attn_mask, _trace=False, _trace_kwargs=None):
    from concourse.bass_utils import run_bass_kernel_spmd

    in_maps, KLE = shard_inputs(query, key, value, Wq, Wk, Wv, Wo, attn_mask)
    nc = _get_nc(KLE)
    res = run_bass_kernel_spmd(
        nc, in_maps, list(range(8)), trace=_trace, **(_trace_kwargs or {})
    )
    out = np.empty((B, Q, D), dtype=np.float32)
    for c in range(8):
        b, half = c // 2, c % 2
        out[b, half * QS:(half + 1) * QS] = res.results[c]["out"]
    if _trace:
        kernel._last_results = res
    return out


# revision 10
# speedup vs baseline: 1.0465x; 1.0465x over previous
"""Multi-head attention Bass/Tile kernel for Trainium2, 8-core SPMD — v3.

v3 over v2: phase D is ACT(exp)-bound, so the PE queue is software-pipelined
(next score tile issues before the current PV pair), the per-head Ln/Exp
normalization is batched into one [8,512] Ln + Exp per query chunk (den rows
gathered by tiny PSUM->SBUF DMAs, reciprocal broadcast via a one-hot selector
matmul), and the K/Q projections for later head-pairs plus the ic0 output
projection are interleaved into the attention loops to fill PE slack.
"""

import sys

if "/opt/trn_rl_repo" not in sys.path:
    sys.path.insert(0, "/opt/trn_rl_repo")

from contextlib import ExitStack

import numpy as np
import ml_dtypes

_BF16NP = ml_dtypes.bfloat16

import concourse.bass as bass
import concourse.tile as tile
from concourse import mybir
import bass_rust as _bass_rust

F32 = mybir.dt.float32
F32R = mybir.dt.float32r
BF16 = mybir.dt.bfloat16
EXP = mybir.ActivationFunctionType.Exp
LN = mybir.ActivationFunctionType.Ln

B, Q, KL, D, H = 4, 2048, 2048, 512, 8
HD = D // H            # 64
QS = Q // 2            # 1024 query rows per core
SCALE = 1.0 / HD ** 0.5
EXPBIAS = -30.0


def _legalize_waits(nc, max_waits=1):
    n = 0
    for f in nc.m.functions:
        for bb in f.blocks:
            insts = bb.instructions
            i = 0
            while i < len(insts):
                inst = insts[i]
                si = inst.sync_info
                if si is not None and len(si.on_wait) > max_waits:
                    waits = list(si.on_wait)
                    for j, w in enumerate(waits[max_waits:]):
                        nop = mybir.InstNoOp(
                            name=f"{inst.name}-waitsplit{j}", ins=[], outs=[]
                        )
                        nop.engine = inst.engine
                        nop.sync_info = _bass_rust.SyncInfo(on_wait=[w], on_update=[])
                        insts.insert(i, nop)
                        i += 1
                        n += 1
                    inst.sync_info = _bass_rust.SyncInfo(
                        on_wait=waits[:max_waits], on_update=list(si.on_update)
                    )
                i += 1
    return n


def build_kernel(KLE):
    NJT = KLE // 128
    nc = bass.Bass("TRN2", target_bir_lowering=False, debug=False)

    qT_d = nc.dram_tensor("qT", [D, QS], F32R, kind="ExternalInput").ap()
    kT_d = nc.dram_tensor("kT", [D, KLE], F32R, kind="ExternalInput").ap()
    vT_d = nc.dram_tensor("vT", [D, KLE], BF16, kind="ExternalInput").ap()
    w_d = {
        w: nc.dram_tensor(w, [D, D], F32R, kind="ExternalInput").ap()
        for w in ("wqT", "wkT", "woT")
    }
    w_d["wvT"] = nc.dram_tensor("wvT", [D, D], BF16, kind="ExternalInput").ap()
    m_d = nc.dram_tensor("mask2d", [128, NJT], F32, kind="ExternalInput").ap()
    out_d = nc.dram_tensor("out", [QS, D], F32, kind="ExternalOutput").ap()

    # one-hot selector: sel[p, hp*128 + m] = 1 if p == 2*hp + (m >= 64)
    sel_np = np.zeros((8, 512), np.float32)
    for hp in range(4):
        sel_np[2 * hp, hp * 128:hp * 128 + 64] = 1.0
        sel_np[2 * hp + 1, hp * 128 + 64:hp * 128 + 128] = 1.0
    sel_d = nc.inline_tensor(sel_np, name="sel")

    with tile.TileContext(nc) as tc, ExitStack() as ctx:
        pc = ctx.enter_context(tc.tile_pool(name="const", bufs=1))
        m_sb = pc.tile([128, NJT], F32, tag="m_sb")
        nc.sync.dma_start(m_sb[:], m_d)
        ebias = pc.tile([128, 1], F32, tag="ebias")
        nc.vector.memset(ebias[:], EXPBIAS)
        sel_f = pc.tile([8, 512], F32, tag="sel_f")
        nc.sync.dma_start(sel_f[:], sel_d.ap())
        sel = pc.tile([8, 512], F32R, tag="sel")
        nc.vector.tensor_copy(sel[:], sel_f[:])

        # ---- input tiles (DMA direct, pre-transposed on host) -----------
        pin = ctx.enter_context(tc.tile_pool(name="inputs", bufs=1))
        wsb = {}
        for w in ("wkT", "wqT", "woT"):
            wsb[w] = [pin.tile([128, D], F32R, tag=f"{w}{i}", name=f"{w}{i}") for i in range(4)]
        wsb["wvT"] = [pin.tile([128, D], BF16, tag=f"wvT{i}", name=f"wvT{i}") for i in range(4)]
        kT = [pin.tile([128, KLE], F32R, tag=f"kTi{i}", name=f"kTi{i}") for i in range(4)]
        qT = [pin.tile([128, QS], F32R, tag=f"qTi{i}", name=f"qTi{i}") for i in range(4)]
        vT = [pin.tile([128, KLE], BF16, tag=f"vTi{i}", name=f"vTi{i}") for i in range(4)]
        # DMA emission order ~ earliest consumer: scores need wq/q-half/wk/k,
        # then the V path streams in per column chunk just ahead of its PV use
        for dk in range(4):
            nc.sync.dma_start(wsb["wqT"][dk][:], w_d["wqT"].rearrange("(t p) d -> t p d", p=128)[dk])
        for dk in range(4):
            nc.sync.dma_start(qT[dk][:, 0:512], qT_d.rearrange("(t p) d -> t p d", p=128)[dk][:, 0:512])
        for dk in range(4):
            nc.sync.dma_start(wsb["wkT"][dk][:], w_d["wkT"].rearrange("(t p) d -> t p d", p=128)[dk])
        for c0 in range(0, KLE, 512):
            cw = min(512, KLE - c0)
            for dk in range(4):
                nc.sync.dma_start(
                    kT[dk][:, c0:c0 + cw],
                    kT_d.rearrange("(t p) d -> t p d", p=128)[dk][:, c0:c0 + cw],
                )
        for dk in range(4):
            nc.sync.dma_start(wsb["wvT"][dk][:], w_d["wvT"].rearrange("(t p) d -> t p d", p=128)[dk])
        for c0 in range(0, KLE, 384):
            cw = min(384, KLE - c0)
            for dk in range(4):
                nc.sync.dma_start(
                    vT[dk][:, c0:c0 + cw],
                    vT_d.rearrange("(t p) d -> t p d", p=128)[dk][:, c0:c0 + cw],
                )
        for dk in range(4):
            nc.sync.dma_start(qT[dk][:, 512:1024], qT_d.rearrange("(t p) d -> t p d", p=128)[dk][:, 512:1024])
        for dk in range(4):
            nc.sync.dma_start(wsb["woT"][dk][:], w_d["woT"].rearrange("(t p) d -> t p d", p=128)[dk])

        pp = ctx.enter_context(tc.tile_pool(name="proj", bufs=1))
        KT = [pp.tile([128, KLE], F32R, tag=f"KT{i}", name=f"KT{i}") for i in range(4)]
        QT = [pp.tile([128, QS], F32R, tag=f"QT{i}", name=f"QT{i}") for i in range(4)]
        VS = [pp.tile([128, H * (HD + 1)], BF16, tag=f"VS{i}", name=f"VS{i}") for i in range(NJT)]
        pA = ctx.enter_context(tc.tile_pool(name="attn_out", bufs=1))
        A2r = [pA.tile([128, QS], F32, tag=f"A2r{hp}", name=f"A2r{hp}") for hp in range(4)]
        A2 = [pA.tile([128, QS], F32R, tag=f"A2{hp}", name=f"A2{hp}") for hp in range(4)]

        with tc.tile_pool(name="eP", bufs=6) as pe_pool, \
             tc.tile_pool(name="rP", bufs=2) as pr, \
             tc.tile_pool(name="bP", bufs=2) as pb, \
             tc.tile_pool(name="psumS", bufs=2, space="PSUM") as ppsS, \
             tc.tile_pool(name="psumPV", bufs=2, space="PSUM") as ppsPV, \
             tc.tile_pool(name="psumX", bufs=2, space="PSUM") as ppsX:

            # ---- deferred work-item machinery (PE filler groups) --------
            nevac = [0]

            def evac(dst, src, eng="v"):
                if eng == "v":
                    nc.vector.tensor_copy(dst, src)
                else:
                    nc.scalar.copy(dst, src)

            # accumulation chains interleaved across two PSUM tiles so one
            # chain's drain overlaps the other's stream
            def _mm_pair(specs):
                tiles = [
                    ppsX.tile([128, 512], F32, tag="aux", name=f"ps_{nm}")
                    for nm, _, _, _ in specs
                ]
                for dk in range(4):
                    for t, (nm, w, lhs_fn, rhs_fn) in zip(tiles, specs):
                        nc.tensor.matmul(
                            t[:, 0:w],
                            lhs_fn(dk), rhs_fn(dk),
                            start=(dk == 0), stop=(dk == 3),
                        )
                return tiles

            def k_spec(ot, j0):
                jw = min(512, KLE - j0)
                return (f"k{ot}_{j0}", jw,
                        lambda dk: wsb["wkT"][dk][:, ot * 128:(ot + 1) * 128],
                        lambda dk: kT[dk][:, j0:j0 + jw])

            def q_spec(ot, icc):
                return (f"q{ot}_{icc}", 512,
                        lambda dk: wsb["wqT"][dk][:, ot * 128:(ot + 1) * 128],
                        lambda dk: qT[dk][:, icc * 512:(icc + 1) * 512])

            def v_spec(jt):
                return (f"v{jt}", 512,
                        lambda dk: vT[dk][:, jt * 128:(jt + 1) * 128],
                        lambda dk: wsb["wvT"][dk][:])

            def k_evac(ot, j0, ps, eng):
                jw = min(512, KLE - j0)
                evac(KT[ot][:, j0:j0 + jw], ps[:, 0:jw], eng)

            def q_evac(ot, icc, ps, eng):
                evac(QT[ot][:, icc * 512:(icc + 1) * 512], ps[:], eng)

            def v_evac(jt, ps):
                vs_out = VS[jt][:].rearrange("p (h d) -> p h d", d=HD + 1)
                nc.vector.tensor_scalar(
                    vs_out[:, :, 0:HD],
                    ps[:].rearrange("p (h d) -> p h d", d=HD),
                    m_sb[:, jt:jt + 1],
                    None,
                    mybir.AluOpType.mult,
                )
                nc.vector.tensor_copy(
                    vs_out[:, :, HD].squeeze(),
                    m_sb[:, jt:jt + 1].broadcast_to([128, H]),
                )

            def wo_group(itl, ic):
                # two query sub-chunks with interleaved accumulation chains
                tiles = []
                for half in range(2):
                    tiles.append(ppsX.tile([128, D], F32, tag="aux", name=f"ops{ic}_{itl}_{half}"))
                for hp in range(4):
                    for half, o_ps in enumerate(tiles):
                        c0 = ic * 512 + itl * 256 + half * 128
                        nc.tensor.matmul(
                            o_ps[:],
                            A2[hp][:, c0:c0 + 128],
                            wsb["woT"][hp][:],
                            start=(hp == 0),
                            stop=(hp == 3),
                        )
                for half, o_ps in enumerate(tiles):
                    c0 = ic * 512 + itl * 256 + half * 128
                    o_sb = pb.tile([128, D], F32, tag="osb", name=f"osb{ic}_{itl}_{half}")
                    nc.vector.tensor_copy(o_sb[:], o_ps[:])
                    nc.sync.dma_start(out_d[c0:c0 + 128, :], o_sb[:])

            # ---- prefix: all Q(ic0) projections run during the kT DMA
            # stall; K0 follows per column-arrival.  V streams in as hp0
            # loop fillers just ahead of its PV consumer.
            t = _mm_pair([q_spec(0, 0), q_spec(1, 0)])
            q_evac(0, 0, t[0], "s")
            q_evac(1, 0, t[1], "v")
            t = _mm_pair([q_spec(2, 0), q_spec(3, 0)])
            q_evac(2, 0, t[0], "s")
            q_evac(3, 0, t[1], "v")
            t = _mm_pair([k_spec(0, 0), k_spec(0, 512)])
            k_evac(0, 0, t[0], "s")
            k_evac(0, 512, t[1], "v")
            if KLE > 1024:
                t = _mm_pair([k_spec(0, 1024)])
                k_evac(0, 1024, t[0], "s")

            # filler queues of (spec, evac) pairs, popped two at a time
            filler = {0: [], 1: []}
            for jt in range(NJT):
                filler[0].append(("v", jt, 0))
            for j0 in range(0, KLE, 512):
                filler[0].append(("k", 1, j0))
            for j0 in range(0, KLE, 512):
                filler[0].append(("k", 2, j0))
            filler[0].append(("q", 1, 1))
            filler[0].append(("q", 2, 1))
            for j0 in range(0, KLE, 512):
                filler[0].append(("k", 3, j0))
            filler[0].append(("q", 3, 1))
            filler[0].append(("q", 0, 1))
            for itl in range(2):
                filler[1].append(("wo", itl, 0))

            def run_filler(ic, hp, jt):
                # V groups must stay ahead of their PV consumer; K/Q groups
                # for ot must land before head-pair ot starts
                pair = []
                while filler[ic] and len(pair) < 2:
                    kind, a, b = filler[ic][0]
                    if kind in ("k", "q") and a > hp + 1:
                        break
                    if kind == "wo" and len(pair) == 1:
                        break  # wo groups emit singly (own psum + dma)
                    pair.append(filler[ic].pop(0))
                    if pair[0][0] == "wo":
                        break
                if not pair:
                    return
                if pair[0][0] == "wo":
                    wo_group(pair[0][1], pair[0][2])
                    return
                specs = []
                for kind, a, b in pair:
                    specs.append(k_spec(a, b) if kind == "k" else
                                 q_spec(a, b) if kind == "q" else v_spec(a))
                ts = _mm_pair(specs)
                for (kind, a, b), ps in zip(pair, ts):
                    if kind == "k":
                        k_evac(a, b, ps, "v")
                    elif kind == "q":
                        q_evac(a, b, ps, "v")
                    else:
                        v_evac(a, ps)

            # ---- attention ----------------------------------------------
            def norm_emit(ic, den_t_, hps, row0):
                # batched reciprocal + broadcast + normalize for `hps`;
                # den rows for hps[k] live at den_t_[row0 + 2k : row0 + 2k+2]
                i0_ = ic * 512
                nr = 2 * len(hps)
                ln_g = pr.tile([8, 512], F32, tag="lng", name=f"lng{ic}_{hps[0]}")
                r_g_ = pr.tile([8, 512], F32R, tag="rg", name=f"rg{ic}_{hps[0]}")
                nc.scalar.activation(ln_g[0:nr, :], den_t_[row0:row0 + nr, :], LN)
                nc.scalar.activation(r_g_[0:nr, :], ln_g[0:nr, :], EXP, scale=-1.0)
                for k, hp_ in enumerate(hps):
                    bc = ppsX.tile([128, 512], F32, tag="aux", name=f"bc{hp_}_{ic}")
                    # sel rows 2k,2k+1 carry the k-th pair's one-hot pattern
                    nc.tensor.matmul(
                        bc[:],
                        sel[0:nr, k * 128:(k + 1) * 128],
                        r_g_[0:nr, :],
                        start=True, stop=True,
                    )
                    nc.vector.tensor_mul(
                        A2[hp_][:, i0_:i0_ + 512], A2r[hp_][:, i0_:i0_ + 512], bc[:]
                    )

            pending_norm = [None]
            for ic in range(2):
                i0 = ic * 512
                den_g = pr.tile([8, 512], F32, tag="deng", name=f"deng{ic}")
                den_g2 = pr.tile([8, 512], F32, tag="deng2", name=f"deng2_{ic}")
                pv_tiles = []
                for hp in range(4):
                    he, ho = 2 * hp, 2 * hp + 1
                    den_t, dr = (den_g2, 0) if (ic == 1 and hp == 3) else (den_g, 2 * hp)
                    pv_e = ppsPV.tile([65, 512], F32, tag="pv", name=f"pve{hp}_{ic}")
                    pv_o = ppsPV.tile([65, 512], F32, tag="pv", name=f"pvo{hp}_{ic}")

                    def s_mm(jt):
                        s_ps = ppsS.tile([128, 1024], F32, tag="s", name=f"s{hp}_{ic}_{jt}")
                        for po2, sl in ((0, slice(0, 512)), (HD, slice(512, 1024))):
                            nc.tensor.matmul(
                                s_ps[:, sl],
                                KT[hp][po2:po2 + HD, jt * 128:(jt + 1) * 128],
                                QT[hp][po2:po2 + HD, i0:i0 + 512],
                                start=True, stop=True,
                            )
                        return s_ps

                    s_cur = s_mm(0)
                    for jt in range(NJT):
                        e_t = pe_pool.tile([128, 1024], BF16, tag="e", name=f"e{hp}_{ic}_{jt}")
                        nc.scalar.activation(e_t[:], s_cur[:], EXP, scale=SCALE, bias=ebias[:, 0:1])
                        if jt + 1 < NJT:
                            s_cur = s_mm(jt + 1)
                        if ic == 1 and hp == 3 and jt == 1 and pending_norm[0] is not None:
                            pending_norm[0]()
                            pending_norm[0] = None
                        if ic == 0 and hp == 0:
                            run_filler(ic, hp, jt)
                        nc.tensor.matmul(
                            pv_e[0:65, :],
                            VS[jt][:, he * (HD + 1):(he + 1) * (HD + 1)],
                            e_t[:, 0:512],
                            start=(jt == 0), stop=(jt == NJT - 1),
                        )
                        nc.tensor.matmul(
                            pv_o[0:65, :],
                            VS[jt][:, ho * (HD + 1):(ho + 1) * (HD + 1)],
                            e_t[:, 512:1024],
                            start=(jt == 0), stop=(jt == NJT - 1),
                        )
                        if not (ic == 0 and hp == 0) and jt % 2 == 1:
                            run_filler(ic, hp, jt)
                    # evacuate raw pair + gather denominators; frees pv slots
                    nc.vector.tensor_copy(A2r[hp][0:HD, i0:i0 + 512], pv_e[0:HD, :])
                    nc.vector.tensor_copy(A2r[hp][HD:128, i0:i0 + 512], pv_o[0:HD, :])
                    # DMA can't read PSUM: bounce den rows through partition 64
                    # of an SBUF stage, then cross-partition SBUF->SBUF DMA
                    stage = pb.tile([65, 1024], F32, tag="dstage", name=f"dst{hp}_{ic}")
                    nc.vector.tensor_copy(stage[64:65, 0:512], pv_e[64:65, :])
                    nc.vector.tensor_copy(stage[64:65, 512:1024], pv_o[64:65, :])
                    nc.sync.dma_start(den_t[dr:dr + 1, :], stage[64:65, 0:512])
                    nc.sync.dma_start(den_t[dr + 1:dr + 2, :], stage[64:65, 512:1024])
                    run_filler(ic, hp, -1)
                    if ic == 1 and hp == 2:
                        # defer hp0-2 normalization into hp3's loop so only
                        # hp3's norm chain sits in the tail
                        pending_norm[0] = (
                            lambda d=den_g: norm_emit(1, d, [0, 1, 2], 0))

                if ic == 0:
                    norm_emit(0, den_g, [0, 1, 2, 3], 0)
            # tail: last pair normalization + second-chunk output projection
            norm_emit(1, den_g2, [3], 0)
            for itl in range(2):
                wo_group(itl, 1)

    return nc


_NC_CACHE = {}


def _get_nc(KLE):
    if KLE not in _NC_CACHE:
        nc = build_kernel(KLE)
        _legalize_waits(nc)
        _NC_CACHE[KLE] = nc
    return _NC_CACHE[KLE]


def shard_inputs(query, key, value, Wq, Wk, Wv, Wo, attn_mask):
    idxs = [np.nonzero(np.asarray(attn_mask[b]) != 0)[0] for b in range(B)]
    maxcnt = max((len(ix) for ix in idxs), default=1)
    KLE = max(128, -(-maxcnt // 128) * 128)
    wqT = np.ascontiguousarray(np.asarray(Wq, np.float32).T)
    wkT = np.ascontiguousarray(np.asarray(Wk, np.float32).T)
    wvT = np.ascontiguousarray(np.asarray(Wv, np.float32).T.astype(_BF16NP))
    woT = np.ascontiguousarray(np.asarray(Wo, np.float32).T)
    in_maps = []
    for c in range(8):
        b, half = c // 2, c % 2
        idx = idxs[b]
        kc = np.zeros((D, KLE), np.float32)
        vc = np.zeros((D, KLE), _BF16NP)
        kc[:, : len(idx)] = np.asarray(key[b], np.float32)[idx].T
        vc[:, : len(idx)] = np.asarray(value[b], np.float32)[idx].T.astype(_BF16NP)
        mf = np.zeros(KLE, np.float32)
        mf[: len(idx)] = 1.0
        in_maps.append({
            "qT": np.ascontiguousarray(
                np.asarray(query[b, half * QS:(half + 1) * QS], np.float32).T
            ),
            "kT": kc,
            "vT": vc,
            "wqT": wqT, "wkT": wkT, "wvT": wvT, "woT": woT,
            "mask2d": np.ascontiguousarray(mf.reshape(KLE // 128, 128).T),
        })
    return in_maps, KLE


def kernel(query, key, value, Wq, Wk, Wv, Wo, attn_mask, _trace=False, _trace_kwargs=None):
    from concourse.bass_utils import run_bass_kernel_spmd

    in_maps, KLE = shard_inputs(query, key, value, Wq, Wk, Wv, Wo, attn_mask)
    nc = _get_nc(KLE)
    res = run_bass_kernel_spmd(
        nc, in_maps, list(range(8)), trace=_trace, **(_trace_kwargs or {})
    )
    out = np.empty((B, Q, D), dtype=np.float32)
    for c in range(8):
        b, half = c // 2, c % 2
        out[b, half * QS:(half + 1) * QS] = res.results[c]["out"]
    if _trace:
        kernel._last_results = res
    return out


# revision 11
# speedup vs baseline: 1.0565x; 1.0096x over previous
"""Multi-head attention Bass/Tile kernel for Trainium2, 8-core SPMD.

Problem: B=4, Q=K=2048, D=512, H=8 heads (head dim 64), fp32.
  head_q = q @ Wq.T ; head_k = k @ Wk.T ; head_v = v @ Wv.T
  S = (head_q . head_k) / 8 ; masked softmax over keys ; out = (P . head_v) @ Wo.T

Sharding: data-parallel over (batch, query-half): core c handles batch c//2,
query rows (c%2)*1024 .. +1024.  Disjoint outputs; no collectives.

Host-side prep (per core): masked kv rows are dropped (softmax over keys is
order-invariant; fully-masked rows contribute exactly zero to numerator and
denominator), survivors packed into KLE rows (KLE = max unmasked count over
batches rounded up to 128; kernel compiled per KLE, cached).  q/k/v and all
weights ship pre-transposed (d-major) so the kernel does no on-chip
transposes; the V path ships bf16 (V enters the output linearly, ~0.2% err).

Device schedule (one core):
  - projections contract over d: stationary = W^T chunk [128,128] f32r,
    moving = x^T [128, cols]; accumulation chains are interleaved in pairs
    across two PSUM tiles so one chain's drain overlaps the other's stream.
  - scores in S^T[j,i] layout; a head pair occupies PSUM partition halves
    and its two K=64 matmuls run concurrently via PE row-tiling, so one
    [128,1024] exp (ScalarE, fused scale+bias) covers both heads.
  - PE queue is software-pipelined: score tile jt+1 issues before the PV
    pair of jt, hiding the exp latency; K/Q/V projections for later head
    pairs stream into the attention loops as filler groups sized to the
    PE slack, with DMA priority order (wq,q-half,wk,k-cols,wv,v-cols,...)
    matching first consumption.
  - the 65th (mask) column of the PV stationary yields the softmax
    denominator free at PSUM partition 64; denominators bounce via an SBUF
    stage row to a packed [8,512] tile, one Ln+Exp batch computes all
    reciprocals for a query chunk, and a one-hot selector matmul broadcasts
    them to the packed A2 pair tile feeding a K=128 Wo contraction.
    ic1's hp0-2 normalization is deferred into hp3's loop so only one norm
    chain remains in the tail.
"""

import sys

if "/opt/trn_rl_repo" not in sys.path:
    sys.path.insert(0, "/opt/trn_rl_repo")

from contextlib import ExitStack

import numpy as np
import ml_dtypes

_BF16NP = ml_dtypes.bfloat16

import concourse.bass as bass
import concourse.tile as tile
from concourse import mybir
import bass_rust as _bass_rust

F32 = mybir.dt.float32
F32R = mybir.dt.float32r
BF16 = mybir.dt.bfloat16
EXP = mybir.ActivationFunctionType.Exp
LN = mybir.ActivationFunctionType.Ln

B, Q, KL, D, H = 4, 2048, 2048, 512, 8
HD = D // H            # 64
QS = Q // 2            # 1024 query rows per core
SCALE = 1.0 / HD ** 0.5
EXPBIAS = -30.0


def _legalize_waits(nc, max_waits=1):
    n = 0
    for f in nc.m.functions:
        for bb in f.blocks:
            insts = bb.instructions
            i = 0
            while i < len(insts):
                inst = insts[i]
                si = inst.sync_info
                if si is not None and len(si.on_wait) > max_waits:
                    waits = list(si.on_wait)
                    for j, w in enumerate(waits[max_waits:]):
                        nop = mybir.InstNoOp(
                            name=f"{inst.name}-waitsplit{j}", ins=[], outs=[]
                        )
                        nop.engine = inst.engine
                        nop.sync_info = _bass_rust.SyncInfo(on_wait=[w], on_update=[])
                        insts.insert(i, nop)
                        i += 1
                        n += 1
                    inst.sync_info = _bass_rust.SyncInfo(
                        on_wait=waits[:max_waits], on_update=list(si.on_update)
                    )
                i += 1
    return n


def build_kernel(KLE):
    NJT = KLE // 128
    nc = bass.Bass("TRN2", target_bir_lowering=False, debug=False)

    qT_d = nc.dram_tensor("qT", [D, QS], F32R, kind="ExternalInput").ap()
    kT_d = nc.dram_tensor("kT", [D, KLE], F32R, kind="ExternalInput").ap()
    vT_d = nc.dram_tensor("vT", [D, KLE], BF16, kind="ExternalInput").ap()
    w_d = {
        w: nc.dram_tensor(w, [D, D], F32R, kind="ExternalInput").ap()
        for w in ("wqT", "wkT", "woT")
    }
    w_d["wvT"] = nc.dram_tensor("wvT", [D, D], BF16, kind="ExternalInput").ap()
    m_d = nc.dram_tensor("mask2d", [128, NJT], F32, kind="ExternalInput").ap()
    out_d = nc.dram_tensor("out", [QS, D], F32, kind="ExternalOutput").ap()

    # one-hot selector: sel[p, hp*128 + m] = 1 if p == 2*hp + (m >= 64)
    sel_np = np.zeros((8, 512), np.float32)
    for hp in range(4):
        sel_np[2 * hp, hp * 128:hp * 128 + 64] = 1.0
        sel_np[2 * hp + 1, hp * 128 + 64:hp * 128 + 128] = 1.0
    sel_d = nc.inline_tensor(sel_np, name="sel")

    with tile.TileContext(nc) as tc, ExitStack() as ctx:
        pc = ctx.enter_context(tc.tile_pool(name="const", bufs=1))
        m_sb = pc.tile([128, NJT], F32, tag="m_sb")
        nc.sync.dma_start(m_sb[:], m_d)
        ebias = pc.tile([128, 1], F32, tag="ebias")
        nc.vector.memset(ebias[:], EXPBIAS)
        sel_f = pc.tile([8, 512], F32, tag="sel_f")
        nc.sync.dma_start(sel_f[:], sel_d.ap())
        sel = pc.tile([8, 512], F32R, tag="sel")
        nc.vector.tensor_copy(sel[:], sel_f[:])

        # ---- input tiles (DMA direct, pre-transposed on host) -----------
        pin = ctx.enter_context(tc.tile_pool(name="inputs", bufs=1))
        wsb = {}
        for w in ("wkT", "wqT", "woT"):
            wsb[w] = [pin.tile([128, D], F32R, tag=f"{w}{i}", name=f"{w}{i}") for i in range(4)]
        wsb["wvT"] = [pin.tile([128, D], BF16, tag=f"wvT{i}", name=f"wvT{i}") for i in range(4)]
        kT = [pin.tile([128, KLE], F32R, tag=f"kTi{i}", name=f"kTi{i}") for i in range(4)]
        qT = [pin.tile([128, QS], F32R, tag=f"qTi{i}", name=f"qTi{i}") for i in range(4)]
        vT = [pin.tile([128, KLE], BF16, tag=f"vTi{i}", name=f"vTi{i}") for i in range(4)]
        # DMA emission order ~ earliest consumer: scores need wq/q-half/wk/k,
        # then the V path streams in per column chunk just ahead of its PV use
        for dk in range(4):
            nc.sync.dma_start(wsb["wqT"][dk][:], w_d["wqT"].rearrange("(t p) d -> t p d", p=128)[dk])
        for dk in range(4):
            nc.sync.dma_start(qT[dk][:, 0:512], qT_d.rearrange("(t p) d -> t p d", p=128)[dk][:, 0:512])
        for dk in range(4):
            nc.sync.dma_start(wsb["wkT"][dk][:], w_d["wkT"].rearrange("(t p) d -> t p d", p=128)[dk])
        for c0 in range(0, KLE, 512):
            cw = min(512, KLE - c0)
            for dk in range(4):
                nc.sync.dma_start(
                    kT[dk][:, c0:c0 + cw],
                    kT_d.rearrange("(t p) d -> t p d", p=128)[dk][:, c0:c0 + cw],
                )
        for dk in range(4):
            nc.sync.dma_start(wsb["wvT"][dk][:], w_d["wvT"].rearrange("(t p) d -> t p d", p=128)[dk])
        for c0 in range(0, KLE, 384):
            cw = min(384, KLE - c0)
            for dk in range(4):
                nc.sync.dma_start(
                    vT[dk][:, c0:c0 + cw],
                    vT_d.rearrange("(t p) d -> t p d", p=128)[dk][:, c0:c0 + cw],
                )
        for dk in range(4):
            nc.sync.dma_start(qT[dk][:, 512:1024], qT_d.rearrange("(t p) d -> t p d", p=128)[dk][:, 512:1024])
        for dk in range(4):
            nc.sync.dma_start(wsb["woT"][dk][:], w_d["woT"].rearrange("(t p) d -> t p d", p=128)[dk])

        pp = ctx.enter_context(tc.tile_pool(name="proj", bufs=1))
        KT = [pp.tile([128, KLE], F32R, tag=f"KT{i}", name=f"KT{i}") for i in range(4)]
        QT = [pp.tile([128, QS], F32R, tag=f"QT{i}", name=f"QT{i}") for i in range(4)]
        VS = [pp.tile([128, H * (HD + 1)], BF16, tag=f"VS{i}", name=f"VS{i}") for i in range(NJT)]
        pA = ctx.enter_context(tc.tile_pool(name="attn_out", bufs=1))
        A2r = [pA.tile([128, QS], F32, tag=f"A2r{hp}", name=f"A2r{hp}") for hp in range(4)]
        A2 = [pA.tile([128, QS], F32R, tag=f"A2{hp}", name=f"A2{hp}") for hp in range(4)]

        with tc.tile_pool(name="eP", bufs=6) as pe_pool, \
             tc.tile_pool(name="rP", bufs=2) as pr, \
             tc.tile_pool(name="bP", bufs=2) as pb, \
             tc.tile_pool(name="psumS", bufs=2, space="PSUM") as ppsS, \
             tc.tile_pool(name="psumPV", bufs=2, space="PSUM") as ppsPV, \
             tc.tile_pool(name="psumX", bufs=2, space="PSUM") as ppsX:

            # ---- deferred work-item machinery (PE filler groups) --------
            nevac = [0]

            def evac(dst, src, eng="v"):
                if eng == "v":
                    nc.vector.tensor_copy(dst, src)
                else:
                    nc.scalar.copy(dst, src)

            # accumulation chains interleaved across two PSUM tiles so one
            # chain's drain overlaps the other's stream
            def _mm_pair(specs):
                tiles = [
                    ppsX.tile([128, 512], F32, tag="aux", name=f"ps_{nm}")
                    for nm, _, _, _ in specs
                ]
                for dk in range(4):
                    for t, (nm, w, lhs_fn, rhs_fn) in zip(tiles, specs):
                        nc.tensor.matmul(
                            t[:, 0:w],
                            lhs_fn(dk), rhs_fn(dk),
                            start=(dk == 0), stop=(dk == 3),
                        )
                return tiles

            def k_spec(ot, j0):
                jw = min(512, KLE - j0)
                return (f"k{ot}_{j0}", jw,
                        lambda dk: wsb["wkT"][dk][:, ot * 128:(ot + 1) * 128],
                        lambda dk: kT[dk][:, j0:j0 + jw])

            def q_spec(ot, icc):
                return (f"q{ot}_{icc}", 512,
                        lambda dk: wsb["wqT"][dk][:, ot * 128:(ot + 1) * 128],
                        lambda dk: qT[dk][:, icc * 512:(icc + 1) * 512])

            def v_spec(jt):
                return (f"v{jt}", 512,
                        lambda dk: vT[dk][:, jt * 128:(jt + 1) * 128],
                        lambda dk: wsb["wvT"][dk][:])

            def k_evac(ot, j0, ps, eng):
                jw = min(512, KLE - j0)
                evac(KT[ot][:, j0:j0 + jw], ps[:, 0:jw], eng)

            def q_evac(ot, icc, ps, eng):
                evac(QT[ot][:, icc * 512:(icc + 1) * 512], ps[:], eng)

            def v_evac(jt, ps):
                vs_out = VS[jt][:].rearrange("p (h d) -> p h d", d=HD + 1)
                nc.vector.tensor_scalar(
                    vs_out[:, :, 0:HD],
                    ps[:].rearrange("p (h d) -> p h d", d=HD),
                    m_sb[:, jt:jt + 1],
                    None,
                    mybir.AluOpType.mult,
                )
                nc.vector.tensor_copy(
                    vs_out[:, :, HD].squeeze(),
                    m_sb[:, jt:jt + 1].broadcast_to([128, H]),
                )

            def wo_group(itl, ic):
                # two query sub-chunks with interleaved accumulation chains
                tiles = []
                for half in range(2):
                    tiles.append(ppsX.tile([128, D], F32, tag="aux", name=f"ops{ic}_{itl}_{half}"))
                for hp in range(4):
                    for half, o_ps in enumerate(tiles):
                        c0 = ic * 512 + itl * 256 + half * 128
                        nc.tensor.matmul(
                            o_ps[:],
                            A2[hp][:, c0:c0 + 128],
                            wsb["woT"][hp][:],
                            start=(hp == 0),
                            stop=(hp == 3),
                        )
                for half, o_ps in enumerate(tiles):
                    c0 = ic * 512 + itl * 256 + half * 128
                    o_sb = pb.tile([128, D], F32, tag="osb", name=f"osb{ic}_{itl}_{half}")
                    nc.vector.tensor_copy(o_sb[:], o_ps[:])
                    nc.sync.dma_start(out_d[c0:c0 + 128, :], o_sb[:])

            # ---- prefix: all Q(ic0) projections run during the kT DMA
            # stall; K0 follows per column-arrival.  V streams in as hp0
            # loop fillers just ahead of its PV consumer.
            t = _mm_pair([q_spec(0, 0), q_spec(1, 0)])
            q_evac(0, 0, t[0], "s")
            q_evac(1, 0, t[1], "v")
            t = _mm_pair([q_spec(2, 0), q_spec(3, 0)])
            q_evac(2, 0, t[0], "s")
            q_evac(3, 0, t[1], "v")
            t = _mm_pair([k_spec(0, 0), k_spec(0, 512)])
            k_evac(0, 0, t[0], "s")
            k_evac(0, 512, t[1], "v")
            if KLE > 1024:
                t = _mm_pair([k_spec(0, 1024)])
                k_evac(0, 1024, t[0], "s")

            # filler queues of (spec, evac) pairs, popped two at a time
            filler = {0: [], 1: []}
            for jt in range(NJT):
                filler[0].append(("v", jt, 0))
            for j0 in range(0, KLE, 512):
                filler[0].append(("k", 1, j0))
            for j0 in range(0, KLE, 512):
                filler[0].append(("k", 2, j0))
            filler[0].append(("q", 1, 1))
            filler[0].append(("q", 2, 1))
            for j0 in range(0, KLE, 512):
                filler[0].append(("k", 3, j0))
            filler[0].append(("q", 3, 1))
            filler[0].append(("q", 0, 1))
            for itl in range(2):
                filler[1].append(("wo", itl, 0))

            def run_filler(ic, hp, jt):
                # V groups must stay ahead of their PV consumer; K/Q groups
                # for ot must land before head-pair ot starts
                pair = []
                while filler[ic] and len(pair) < 2:
                    kind, a, b = filler[ic][0]
                    if kind in ("k", "q") and a > hp + 1:
                        break
                    if kind == "wo" and len(pair) == 1:
                        break  # wo groups emit singly (own psum + dma)
                    pair.append(filler[ic].pop(0))
                    if pair[0][0] == "wo":
                        break
                if not pair:
                    return
                if pair[0][0] == "wo":
                    wo_group(pair[0][1], pair[0][2])
                    return
                specs = []
                for kind, a, b in pair:
                    specs.append(k_spec(a, b) if kind == "k" else
                                 q_spec(a, b) if kind == "q" else v_spec(a))
                ts = _mm_pair(specs)
                for (kind, a, b), ps in zip(pair, ts):
                    if kind == "k":
                        k_evac(a, b, ps, "v")
                    elif kind == "q":
                        q_evac(a, b, ps, "v")
                    else:
                        v_evac(a, ps)

            # ---- attention ----------------------------------------------
            def norm_emit(ic, den_t_, hps, row0):
                # batched reciprocal + broadcast + normalize for `hps`;
                # den rows for hps[k] live at den_t_[row0 + 2k : row0 + 2k+2]
                i0_ = ic * 512
                nr = 2 * len(hps)
                ln_g = pr.tile([8, 512], F32, tag="lng", name=f"lng{ic}_{hps[0]}")
                r_g_ = pr.tile([8, 512], F32R, tag="rg", name=f"rg{ic}_{hps[0]}")
                nc.scalar.activation(ln_g[0:nr, :], den_t_[row0:row0 + nr, :], LN)
                nc.scalar.activation(r_g_[0:nr, :], ln_g[0:nr, :], EXP, scale=-1.0)
                for k, hp_ in enumerate(hps):
                    bc = ppsX.tile([128, 512], F32, tag="aux", name=f"bc{hp_}_{ic}")
                    # sel rows 2k,2k+1 carry the k-th pair's one-hot pattern
                    nc.tensor.matmul(
                        bc[:],
                        sel[0:nr, k * 128:(k + 1) * 128],
                        r_g_[0:nr, :],
                        start=True, stop=True,
                    )
                    nc.vector.tensor_mul(
                        A2[hp_][:, i0_:i0_ + 512], A2r[hp_][:, i0_:i0_ + 512], bc[:]
                    )

            pending_norm = [None]
            for ic in range(2):
                i0 = ic * 512
                den_g = pr.tile([8, 512], F32, tag="deng", name=f"deng{ic}")
                den_g2 = pr.tile([8, 512], F32, tag="deng2", name=f"deng2_{ic}")
                pv_tiles = []
                for hp in range(4):
                    he, ho = 2 * hp, 2 * hp + 1
                    den_t, dr = (den_g2, 0) if (ic == 1 and hp == 3) else (den_g, 2 * hp)
                    pv_e = ppsPV.tile([65, 512], F32, tag="pv", name=f"pve{hp}_{ic}")
                    pv_o = ppsPV.tile([65, 512], F32, tag="pv", name=f"pvo{hp}_{ic}")

                    def s_mm(jt):
                        s_ps = ppsS.tile([128, 1024], F32, tag="s", name=f"s{hp}_{ic}_{jt}")
                        for po2, sl in ((0, slice(0, 512)), (HD, slice(512, 1024))):
                            nc.tensor.matmul(
                                s_ps[:, sl],
                                KT[hp][po2:po2 + HD, jt * 128:(jt + 1) * 128],
                                QT[hp][po2:po2 + HD, i0:i0 + 512],
                                start=True, stop=True,
                            )
                        return s_ps

                    s_cur = s_mm(0)
                    for jt in range(NJT):
                        e_t = pe_pool.tile([128, 1024], BF16, tag="e", name=f"e{hp}_{ic}_{jt}")
                        nc.scalar.activation(e_t[:], s_cur[:], EXP, scale=SCALE, bias=ebias[:, 0:1])
                        if jt + 1 < NJT:
                            s_cur = s_mm(jt + 1)
                        if ic == 1 and hp == 3 and jt == 1 and pending_norm[0] is not None:
                            pending_norm[0]()
                            pending_norm[0] = None
                        if ic == 0 and hp == 0:
                            run_filler(ic, hp, jt)
                        nc.tensor.matmul(
                            pv_e[0:65, :],
                            VS[jt][:, he * (HD + 1):(he + 1) * (HD + 1)],
                            e_t[:, 0:512],
                            start=(jt == 0), stop=(jt == NJT - 1),
                        )
                        nc.tensor.matmul(
                            pv_o[0:65, :],
                            VS[jt][:, ho * (HD + 1):(ho + 1) * (HD + 1)],
                            e_t[:, 512:1024],
                            start=(jt == 0), stop=(jt == NJT - 1),
                        )
                        if not (ic == 0 and hp == 0) and jt % 2 == 1:
                            run_filler(ic, hp, jt)
                    # evacuate raw pair + gather denominators; frees pv slots
                    nc.vector.tensor_copy(A2r[hp][0:HD, i0:i0 + 512], pv_e[0:HD, :])
                    nc.vector.tensor_copy(A2r[hp][HD:128, i0:i0 + 512], pv_o[0:HD, :])
                    # DMA can't read PSUM: bounce den rows through partition 64
                    # of an SBUF stage, then cross-partition SBUF->SBUF DMA
                    stage = pb.tile([65, 1024], F32, tag="dstage", name=f"dst{hp}_{ic}")
                    nc.vector.tensor_copy(stage[64:65, 0:512], pv_e[64:65, :])
                    nc.vector.tensor_copy(stage[64:65, 512:1024], pv_o[64:65, :])
                    nc.sync.dma_start(den_t[dr:dr + 1, :], stage[64:65, 0:512])
                    nc.sync.dma_start(den_t[dr + 1:dr + 2, :], stage[64:65, 512:1024])
                    run_filler(ic, hp, -1)
                    if ic == 1 and hp == 2:
                        # defer hp0-2 normalization into hp3's loop so only
                        # hp3's norm chain sits in the tail
                        pending_norm[0] = (
                            lambda d=den_g: norm_emit(1, d, [0, 1, 2], 0))

                if ic == 0:
                    norm_emit(0, den_g, [0, 1, 2, 3], 0)
            # tail: last pair normalization + second-chunk output projection
            norm_emit(1, den_g2, [3], 0)
            for itl in range(2):
                wo_group(itl, 1)

    return nc


_NC_CACHE = {}


def _get_nc(KLE):
    if KLE not in _NC_CACHE:
        nc = build_kernel(KLE)
        _legalize_waits(nc)
        _NC_CACHE[KLE] = nc
    return _NC_CACHE[KLE]


def shard_inputs(query, key, value, Wq, Wk, Wv, Wo, attn_mask):
    idxs = [np.nonzero(np.asarray(attn_mask[b]) != 0)[0] for b in range(B)]
    maxcnt = max((len(ix) for ix in idxs), default=1)
    KLE = max(128, -(-maxcnt // 128) * 128)
    wqT = np.ascontiguousarray(np.asarray(Wq, np.float32).T)
    wkT = np.ascontiguousarray(np.asarray(Wk, np.float32).T)
    wvT = np.ascontiguousarray(np.asarray(Wv, np.float32).T.astype(_BF16NP))
    woT = np.ascontiguousarray(np.asarray(Wo, np.float32).T)
    in_maps = []
    for c in range(8):
        b, half = c // 2, c % 2
        idx = idxs[b]
        kc = np.zeros((D, KLE), np.float32)
        vc = np.zeros((D, KLE), _BF16NP)
        kc[:, : len(idx)] = np.asarray(key[b], np.float32)[idx].T
        vc[:, : len(idx)] = np.asarray(value[b], np.float32)[idx].T.astype(_BF16NP)
        mf = np.zeros(KLE, np.float32)
        mf[: len(idx)] = 1.0
        in_maps.append({
            "qT": np.ascontiguousarray(
                np.asarray(query[b, half * QS:(half + 1) * QS], np.float32).T
            ),
            "kT": kc,
            "vT": vc,
            "wqT": wqT, "wkT": wkT, "wvT": wvT, "woT": woT,
            "mask2d": np.ascontiguousarray(mf.reshape(KLE // 128, 128).T),
        })
    return in_maps, KLE


def kernel(query, key, value, Wq, Wk, Wv, Wo, attn_mask, _trace=False, _trace_kwargs=None):
    from concourse.bass_utils import run_bass_kernel_spmd

    in_maps, KLE = shard_inputs(query, key, value, Wq, Wk, Wv, Wo, attn_mask)
    nc = _get_nc(KLE)
    res = run_bass_kernel_spmd(
        nc, in_maps, list(range(8)), trace=_trace, **(_trace_kwargs or {})
    )
    out = np.empty((B, Q, D), dtype=np.float32)
    for c in range(8):
        b, half = c // 2, c % 2
        out[b, half * QS:(half + 1) * QS] = res.results[c]["out"]
    if _trace:
        kernel._last_results = res
    return out


# revision 12
# speedup vs baseline: 1.0572x; 1.0006x over previous
"""Multi-head attention Bass/Tile kernel for Trainium2, 8-core SPMD.

Problem: B=4, Q=K=2048, D=512, H=8 heads (head dim 64), fp32.
  head_q = q @ Wq.T ; head_k = k @ Wk.T ; head_v = v @ Wv.T
  S = (head_q . head_k) / 8 ; masked softmax over keys ; out = (P . head_v) @ Wo.T

Sharding: data-parallel over (batch, query-half): core c handles batch c//2,
query rows (c%2)*1024 .. +1024.  Disjoint outputs; no collectives.

Host-side prep (per core): masked kv rows are dropped (softmax over keys is
order-invariant; fully-masked rows contribute exactly zero to numerator and
denominator), survivors packed into KLE rows (KLE = max unmasked count over
batches rounded up to 128; kernel compiled per KLE, cached).  q/k/v and all
weights ship pre-transposed (d-major) so the kernel does no on-chip
transposes; the V path ships bf16 (V enters the output linearly, ~0.2% err).

Device schedule (one core):
  - projections contract over d: stationary = W^T chunk [128,128] f32r,
    moving = x^T [128, cols]; accumulation chains are interleaved in pairs
    across two PSUM tiles so one chain's drain overlaps the other's stream.
  - scores in S^T[j,i] layout; a head pair occupies PSUM partition halves
    and its two K=64 matmuls run concurrently via PE row-tiling, so one
    [128,1024] exp (ScalarE, fused scale+bias) covers both heads.
  - PE queue is software-pipelined: score tile jt+1 issues before the PV
    pair of jt, hiding the exp latency; K/Q/V projections for later head
    pairs stream into the attention loops as filler groups sized to the
    PE slack, with DMA priority order (wq,q-half,wk,k-cols,wv,v-cols,...)
    matching first consumption.
  - the 65th (mask) column of the PV stationary yields the softmax
    denominator free at PSUM partition 64; denominators bounce via an SBUF
    stage row to a packed [8,512] tile, one Ln+Exp batch computes all
    reciprocals for a query chunk, and a one-hot selector matmul broadcasts
    them to the packed A2 pair tile feeding a K=128 Wo contraction.
    ic1's hp0-2 normalization is deferred into hp3's loop so only one norm
    chain remains in the tail.
"""

import sys

if "/opt/trn_rl_repo" not in sys.path:
    sys.path.insert(0, "/opt/trn_rl_repo")

from contextlib import ExitStack

import numpy as np
import ml_dtypes

_BF16NP = ml_dtypes.bfloat16

import concourse.bass as bass
import concourse.tile as tile
from concourse import mybir
import bass_rust as _bass_rust

F32 = mybir.dt.float32
F32R = mybir.dt.float32r
BF16 = mybir.dt.bfloat16
EXP = mybir.ActivationFunctionType.Exp
LN = mybir.ActivationFunctionType.Ln

B, Q, KL, D, H = 4, 2048, 2048, 512, 8
HD = D // H            # 64
QS = Q // 2            # 1024 query rows per core
SCALE = 1.0 / HD ** 0.5
EXPBIAS = -30.0


def _legalize_waits(nc, max_waits=1):
    n = 0
    for f in nc.m.functions:
        for bb in f.blocks:
            insts = bb.instructions
            i = 0
            while i < len(insts):
                inst = insts[i]
                si = inst.sync_info
                if si is not None and len(si.on_wait) > max_waits:
                    waits = list(si.on_wait)
                    for j, w in enumerate(waits[max_waits:]):
                        nop = mybir.InstNoOp(
                            name=f"{inst.name}-waitsplit{j}", ins=[], outs=[]
                        )
                        nop.engine = inst.engine
                        nop.sync_info = _bass_rust.SyncInfo(on_wait=[w], on_update=[])
                        insts.insert(i, nop)
                        i += 1
                        n += 1
                    inst.sync_info = _bass_rust.SyncInfo(
                        on_wait=waits[:max_waits], on_update=list(si.on_update)
                    )
                i += 1
    return n


def build_kernel(KLE):
    NJT = KLE // 128
    nc = bass.Bass("TRN2", target_bir_lowering=False, debug=False)

    qT_d = nc.dram_tensor("qT", [D, QS], F32R, kind="ExternalInput").ap()
    kT_d = nc.dram_tensor("kT", [D, KLE], F32R, kind="ExternalInput").ap()
    vT_d = nc.dram_tensor("vT", [D, KLE], BF16, kind="ExternalInput").ap()
    w_d = {
        w: nc.dram_tensor(w, [D, D], F32R, kind="ExternalInput").ap()
        for w in ("wqT", "wkT", "woT")
    }
    w_d["wvT"] = nc.dram_tensor("wvT", [D, D], BF16, kind="ExternalInput").ap()
    m_d = nc.dram_tensor("mask2d", [128, NJT], F32, kind="ExternalInput").ap()
    out_d = nc.dram_tensor("out", [QS, D], F32, kind="ExternalOutput").ap()

    # one-hot selector: sel[p, hp*128 + m] = 1 if p == 2*hp + (m >= 64)
    sel_np = np.zeros((8, 512), np.float32)
    for hp in range(4):
        sel_np[2 * hp, hp * 128:hp * 128 + 64] = 1.0
        sel_np[2 * hp + 1, hp * 128 + 64:hp * 128 + 128] = 1.0
    sel_d = nc.inline_tensor(sel_np, name="sel")

    with tile.TileContext(nc) as tc, ExitStack() as ctx:
        pc = ctx.enter_context(tc.tile_pool(name="const", bufs=1))
        m_sb = pc.tile([128, NJT], F32, tag="m_sb")
        nc.sync.dma_start(m_sb[:], m_d)
        ebias = pc.tile([128, 1], F32, tag="ebias")
        nc.vector.memset(ebias[:], EXPBIAS)
        sel_f = pc.tile([8, 512], F32, tag="sel_f")
        nc.sync.dma_start(sel_f[:], sel_d.ap())
        sel = pc.tile([8, 512], F32R, tag="sel")
        nc.vector.tensor_copy(sel[:], sel_f[:])

        # ---- input tiles (DMA direct, pre-transposed on host) -----------
        pin = ctx.enter_context(tc.tile_pool(name="inputs", bufs=1))
        wsb = {}
        for w in ("wkT", "wqT", "woT"):
            wsb[w] = [pin.tile([128, D], F32R, tag=f"{w}{i}", name=f"{w}{i}") for i in range(4)]
        wsb["wvT"] = [pin.tile([128, D], BF16, tag=f"wvT{i}", name=f"wvT{i}") for i in range(4)]
        kT = [pin.tile([128, KLE], F32R, tag=f"kTi{i}", name=f"kTi{i}") for i in range(4)]
        qT = [pin.tile([128, QS], F32R, tag=f"qTi{i}", name=f"qTi{i}") for i in range(4)]
        vT = [pin.tile([128, KLE], BF16, tag=f"vTi{i}", name=f"vTi{i}") for i in range(4)]
        # DMA emission order ~ earliest consumer: scores need wq/q-half/wk/k,
        # then the V path streams in per column chunk just ahead of its PV use
        for dk in range(4):
            nc.sync.dma_start(wsb["wqT"][dk][:], w_d["wqT"].rearrange("(t p) d -> t p d", p=128)[dk])
        for dk in range(4):
            nc.sync.dma_start(qT[dk][:, 0:512], qT_d.rearrange("(t p) d -> t p d", p=128)[dk][:, 0:512])
        for dk in range(4):
            nc.sync.dma_start(wsb["wkT"][dk][:], w_d["wkT"].rearrange("(t p) d -> t p d", p=128)[dk])
        for c0 in range(0, KLE, 512):
            cw = min(512, KLE - c0)
            for dk in range(4):
                nc.sync.dma_start(
                    kT[dk][:, c0:c0 + cw],
                    kT_d.rearrange("(t p) d -> t p d", p=128)[dk][:, c0:c0 + cw],
                )
        for dk in range(4):
            nc.sync.dma_start(wsb["wvT"][dk][:], w_d["wvT"].rearrange("(t p) d -> t p d", p=128)[dk])
        for c0 in range(0, KLE, 384):
            cw = min(384, KLE - c0)
            for dk in range(4):
                nc.sync.dma_start(
                    vT[dk][:, c0:c0 + cw],
                    vT_d.rearrange("(t p) d -> t p d", p=128)[dk][:, c0:c0 + cw],
                )
        for dk in range(4):
            nc.sync.dma_start(qT[dk][:, 512:1024], qT_d.rearrange("(t p) d -> t p d", p=128)[dk][:, 512:1024])
        for dk in range(4):
            nc.sync.dma_start(wsb["woT"][dk][:], w_d["woT"].rearrange("(t p) d -> t p d", p=128)[dk])

        pp = ctx.enter_context(tc.tile_pool(name="proj", bufs=1))
        KT = [pp.tile([128, KLE], F32R, tag=f"KT{i}", name=f"KT{i}") for i in range(4)]
        QT = [pp.tile([128, QS], F32R, tag=f"QT{i}", name=f"QT{i}") for i in range(4)]
        VS = [pp.tile([128, H * (HD + 1)], BF16, tag=f"VS{i}", name=f"VS{i}") for i in range(NJT)]
        pA = ctx.enter_context(tc.tile_pool(name="attn_out", bufs=1))
        A2r = [pA.tile([128, QS], F32, tag=f"A2r{hp}", name=f"A2r{hp}") for hp in range(4)]
        A2 = [pA.tile([128, QS], F32R, tag=f"A2{hp}", name=f"A2{hp}") for hp in range(4)]

        with tc.tile_pool(name="eP", bufs=6) as pe_pool, \
             tc.tile_pool(name="rP", bufs=2) as pr, \
             tc.tile_pool(name="bP", bufs=2) as pb, \
             tc.tile_pool(name="psumS", bufs=2, space="PSUM") as ppsS, \
             tc.tile_pool(name="psumPV", bufs=2, space="PSUM") as ppsPV, \
             tc.tile_pool(name="psumX", bufs=2, space="PSUM") as ppsX:

            # ---- deferred work-item machinery (PE filler groups) --------
            nevac = [0]

            def evac(dst, src, eng="v"):
                if eng == "v":
                    nc.vector.tensor_copy(dst, src)
                else:
                    nc.scalar.copy(dst, src)

            # accumulation chains interleaved across two PSUM tiles so one
            # chain's drain overlaps the other's stream
            def _mm_pair(specs):
                tiles = [
                    ppsX.tile([128, 512], F32, tag="aux", name=f"ps_{nm}")
                    for nm, _, _, _ in specs
                ]
                for dk in range(4):
                    for t, (nm, w, lhs_fn, rhs_fn) in zip(tiles, specs):
                        nc.tensor.matmul(
                            t[:, 0:w],
                            lhs_fn(dk), rhs_fn(dk),
                            start=(dk == 0), stop=(dk == 3),
                        )
                return tiles

            def k_spec(ot, j0):
                jw = min(512, KLE - j0)
                return (f"k{ot}_{j0}", jw,
                        lambda dk: wsb["wkT"][dk][:, ot * 128:(ot + 1) * 128],
                        lambda dk: kT[dk][:, j0:j0 + jw])

            def q_spec(ot, icc):
                return (f"q{ot}_{icc}", 512,
                        lambda dk: wsb["wqT"][dk][:, ot * 128:(ot + 1) * 128],
                        lambda dk: qT[dk][:, icc * 512:(icc + 1) * 512])

            def v_spec(jt):
                return (f"v{jt}", 512,
                        lambda dk: vT[dk][:, jt * 128:(jt + 1) * 128],
                        lambda dk: wsb["wvT"][dk][:])

            def k_evac(ot, j0, ps, eng):
                jw = min(512, KLE - j0)
                evac(KT[ot][:, j0:j0 + jw], ps[:, 0:jw], eng)

            def q_evac(ot, icc, ps, eng):
                evac(QT[ot][:, icc * 512:(icc + 1) * 512], ps[:], eng)

            def v_evac(jt, ps):
                vs_out = VS[jt][:].rearrange("p (h d) -> p h d", d=HD + 1)
                nc.vector.tensor_scalar(
                    vs_out[:, :, 0:HD],
                    ps[:].rearrange("p (h d) -> p h d", d=HD),
                    m_sb[:, jt:jt + 1],
                    None,
                    mybir.AluOpType.mult,
                )
                nc.vector.tensor_copy(
                    vs_out[:, :, HD].squeeze(),
                    m_sb[:, jt:jt + 1].broadcast_to([128, H]),
                )

            def wo_group(itl, ic):
                # two query sub-chunks with interleaved accumulation chains
                tiles = []
                for half in range(2):
                    tiles.append(ppsX.tile([128, D], F32, tag="aux", name=f"ops{ic}_{itl}_{half}"))
                for hp in range(4):
                    for half, o_ps in enumerate(tiles):
                        c0 = ic * 512 + itl * 256 + half * 128
                        nc.tensor.matmul(
                            o_ps[:],
                            A2[hp][:, c0:c0 + 128],
                            wsb["woT"][hp][:],
                            start=(hp == 0),
                            stop=(hp == 3),
                        )
                for half, o_ps in enumerate(tiles):
                    c0 = ic * 512 + itl * 256 + half * 128
                    o_sb = pb.tile([128, D], F32, tag="osb", name=f"osb{ic}_{itl}_{half}")
                    nc.vector.tensor_copy(o_sb[:], o_ps[:])
                    nc.sync.dma_start(out_d[c0:c0 + 128, :], o_sb[:])

            # ---- prefix: all Q(ic0) projections run during the kT DMA
            # stall; K0 follows per column-arrival.  V streams in as hp0
            # loop fillers just ahead of its PV consumer.
            t = _mm_pair([q_spec(0, 0), q_spec(1, 0)])
            q_evac(0, 0, t[0], "s")
            q_evac(1, 0, t[1], "v")
            t = _mm_pair([q_spec(2, 0), q_spec(3, 0)])
            q_evac(2, 0, t[0], "s")
            q_evac(3, 0, t[1], "v")
            t = _mm_pair([k_spec(0, 0), k_spec(0, 512)])
            k_evac(0, 0, t[0], "s")
            k_evac(0, 512, t[1], "v")
            if KLE > 1024:
                t = _mm_pair([k_spec(0, 1024)])
                k_evac(0, 1024, t[0], "s")

            # filler queues of (spec, evac) pairs, popped two at a time
            filler = {0: [], 1: []}
            for jt in range(NJT):
                filler[0].append(("v", jt, 0))
            for j0 in range(0, KLE, 512):
                filler[0].append(("k", 1, j0))
            for j0 in range(0, KLE, 512):
                filler[0].append(("k", 2, j0))
            filler[0].append(("q", 1, 1))
            filler[0].append(("q", 2, 1))
            for j0 in range(0, KLE, 512):
                filler[0].append(("k", 3, j0))
            filler[0].append(("q", 3, 1))
            filler[0].append(("q", 0, 1))
            for itl in range(2):
                filler[1].append(("wo", itl, 0))

            def run_filler(ic, hp, jt):
                # V groups must stay ahead of their PV consumer; K/Q groups
                # for ot must land before head-pair ot starts; wo needs the
                # deferred ic0 normalization to have been emitted
                if filler[ic] and filler[ic][0][0] == "wo" and 0 <= jt < 3:
                    return
                pair = []
                while filler[ic] and len(pair) < 2:
                    kind, a, b = filler[ic][0]
                    if kind in ("k", "q") and a > hp + 1:
                        break
                    if kind == "wo" and len(pair) == 1:
                        break  # wo groups emit singly (own psum + dma)
                    pair.append(filler[ic].pop(0))
                    if pair[0][0] == "wo":
                        break
                if not pair:
                    return
                if pair[0][0] == "wo":
                    wo_group(pair[0][1], pair[0][2])
                    return
                specs = []
                for kind, a, b in pair:
                    specs.append(k_spec(a, b) if kind == "k" else
                                 q_spec(a, b) if kind == "q" else v_spec(a))
                ts = _mm_pair(specs)
                for (kind, a, b), ps in zip(pair, ts):
                    if kind == "k":
                        k_evac(a, b, ps, "v")
                    elif kind == "q":
                        q_evac(a, b, ps, "v")
                    else:
                        v_evac(a, ps)

            # ---- attention ----------------------------------------------
            def norm_emit(ic, den_t_, hps, row0):
                # batched reciprocal + broadcast + normalize for `hps`;
                # den rows for hps[k] live at den_t_[row0 + 2k : row0 + 2k+2]
                i0_ = ic * 512
                nr = 2 * len(hps)
                ln_g = pr.tile([8, 512], F32, tag="lng", name=f"lng{ic}_{hps[0]}")
                r_g_ = pr.tile([8, 512], F32R, tag="rg", name=f"rg{ic}_{hps[0]}")
                nc.scalar.activation(ln_g[0:nr, :], den_t_[row0:row0 + nr, :], LN)
                nc.scalar.activation(r_g_[0:nr, :], ln_g[0:nr, :], EXP, scale=-1.0)
                for k, hp_ in enumerate(hps):
                    bc = ppsX.tile([128, 512], F32, tag="aux", name=f"bc{hp_}_{ic}")
                    # sel rows 2k,2k+1 carry the k-th pair's one-hot pattern
                    nc.tensor.matmul(
                        bc[:],
                        sel[0:nr, k * 128:(k + 1) * 128],
                        r_g_[0:nr, :],
                        start=True, stop=True,
                    )
                    nc.vector.tensor_mul(
                        A2[hp_][:, i0_:i0_ + 512], A2r[hp_][:, i0_:i0_ + 512], bc[:]
                    )

            pending_norm = [None]
            for ic in range(2):
                i0 = ic * 512
                den_g = pr.tile([8, 512], F32, tag="deng", name=f"deng{ic}")
                den_g2 = pr.tile([8, 512], F32, tag="deng2", name=f"deng2_{ic}")
                pv_tiles = []
                for hp in range(4):
                    he, ho = 2 * hp, 2 * hp + 1
                    den_t, dr = (den_g2, 0) if (ic == 1 and hp == 3) else (den_g, 2 * hp)
                    pv_e = ppsPV.tile([65, 512], F32, tag="pv", name=f"pve{hp}_{ic}")
                    pv_o = ppsPV.tile([65, 512], F32, tag="pv", name=f"pvo{hp}_{ic}")

                    def s_mm(jt):
                        s_ps = ppsS.tile([128, 1024], F32, tag="s", name=f"s{hp}_{ic}_{jt}")
                        for po2, sl in ((0, slice(0, 512)), (HD, slice(512, 1024))):
                            nc.tensor.matmul(
                                s_ps[:, sl],
                                KT[hp][po2:po2 + HD, jt * 128:(jt + 1) * 128],
                                QT[hp][po2:po2 + HD, i0:i0 + 512],
                                start=True, stop=True,
                            )
                        return s_ps

                    s_cur = s_mm(0)
                    for jt in range(NJT):
                        e_t = pe_pool.tile([128, 1024], BF16, tag="e", name=f"e{hp}_{ic}_{jt}")
                        nc.scalar.activation(e_t[:], s_cur[:], EXP, scale=SCALE, bias=ebias[:, 0:1])
                        if jt + 1 < NJT:
                            s_cur = s_mm(jt + 1)
                        if jt == 1 and pending_norm[0] is not None:
                            pending_norm[0]()
                            pending_norm[0] = None
                        if ic == 0 and hp == 0:
                            run_filler(ic, hp, jt)
                        nc.tensor.matmul(
                            pv_e[0:65, :],
                            VS[jt][:, he * (HD + 1):(he + 1) * (HD + 1)],
                            e_t[:, 0:512],
                            start=(jt == 0), stop=(jt == NJT - 1),
                        )
                        nc.tensor.matmul(
                            pv_o[0:65, :],
                            VS[jt][:, ho * (HD + 1):(ho + 1) * (HD + 1)],
                            e_t[:, 512:1024],
                            start=(jt == 0), stop=(jt == NJT - 1),
                        )
                        if not (ic == 0 and hp == 0) and jt % 2 == 1:
                            run_filler(ic, hp, jt)
                    # evacuate raw pair + gather denominators; frees pv slots
                    nc.vector.tensor_copy(A2r[hp][0:HD, i0:i0 + 512], pv_e[0:HD, :])
                    nc.vector.tensor_copy(A2r[hp][HD:128, i0:i0 + 512], pv_o[0:HD, :])
                    # DMA can't read PSUM: bounce den rows through partition 64
                    # of an SBUF stage, then cross-partition SBUF->SBUF DMA
                    stage = pb.tile([65, 1024], F32, tag="dstage", name=f"dst{hp}_{ic}")
                    nc.vector.tensor_copy(stage[64:65, 0:512], pv_e[64:65, :])
                    nc.vector.tensor_copy(stage[64:65, 512:1024], pv_o[64:65, :])
                    nc.sync.dma_start(den_t[dr:dr + 1, :], stage[64:65, 0:512])
                    nc.sync.dma_start(den_t[dr + 1:dr + 2, :], stage[64:65, 512:1024])
                    run_filler(ic, hp, -1)
                    if ic == 1 and hp == 2:
                        # defer hp0-2 normalization into hp3's loop so only
                        # hp3's norm chain sits in the tail
                        pending_norm[0] = (
                            lambda d=den_g: norm_emit(1, d, [0, 1, 2], 0))

                if ic == 0:
                    pending_norm[0] = (
                        lambda d=den_g: norm_emit(0, d, [0, 1, 2, 3], 0))
            # tail: last pair normalization + second-chunk output projection
            norm_emit(1, den_g2, [3], 0)
            for itl in range(2):
                wo_group(itl, 1)

    return nc


_NC_CACHE = {}


def _get_nc(KLE):
    if KLE not in _NC_CACHE:
        nc = build_kernel(KLE)
        _legalize_waits(nc)
        _NC_CACHE[KLE] = nc
    return _NC_CACHE[KLE]


def shard_inputs(query, key, value, Wq, Wk, Wv, Wo, attn_mask):
    idxs = [np.nonzero(np.asarray(attn_mask[b]) != 0)[0] for b in range(B)]
    maxcnt = max((len(ix) for ix in idxs), default=1)
    KLE = max(128, -(-maxcnt // 128) * 128)
    wqT = np.ascontiguousarray(np.asarray(Wq, np.float32).T)
    wkT = np.ascontiguousarray(np.asarray(Wk, np.float32).T)
    wvT = np.ascontiguousarray(np.asarray(Wv, np.float32).T.astype(_BF16NP))
    woT = np.ascontiguousarray(np.asarray(Wo, np.float32).T)
    in_maps = []
    for c in range(8):
        b, half = c // 2, c % 2
        idx = idxs[b]
        kc = np.zeros((D, KLE), np.float32)
        vc = np.zeros((D, KLE), _BF16NP)
        kc[:, : len(idx)] = np.asarray(key[b], np.float32)[idx].T
        vc[:, : len(idx)] = np.asarray(value[b], np.float32)[idx].T.astype(_BF16NP)
        mf = np.zeros(KLE, np.float32)
        mf[: len(idx)] = 1.0
        in_maps.append({
            "qT": np.ascontiguousarray(
                np.asarray(query[b, half * QS:(half + 1) * QS], np.float32).T
            ),
            "kT": kc,
            "vT": vc,
            "wqT": wqT, "wkT": wkT, "wvT": wvT, "woT": woT,
            "mask2d": np.ascontiguousarray(mf.reshape(KLE // 128, 128).T),
        })
    return in_maps, KLE


def kernel(query, key, value, Wq, Wk, Wv, Wo, attn_mask, _trace=False, _trace_kwargs=None):
    from concourse.bass_utils import run_bass_kernel_spmd

    in_maps, KLE = shard_inputs(query, key, value, Wq, Wk, Wv, Wo, attn_mask)
    nc = _get_nc(KLE)
    res = run_bass_kernel_spmd(
        nc, in_maps, list(range(8)), trace=_trace, **(_trace_kwargs or {})
    )
    out = np.empty((B, Q, D), dtype=np.float32)
    for c in range(8):
        b, half = c // 2, c % 2
        out[b, half * QS:(half + 1) * QS] = res.results[c]["out"]
    if _trace:
        kernel._last_results = res
    return out
